# revision 1
# baseline (speedup 1.0000x reference)
"""Trainium2 Bass kernel for the Expected-Depth DP loss.

Computation (see reference):
  - edge_max = max over first 7 of 8 op-logits          [S, 64, 16]
  - w        = masked softmax over the 16-wide window   [S, 64, 16]
  - DP scan:  ed[j] = sum_k w[j,k] * (ed[base+k] + 1),  j = 2..65
  - loss     = sum_s theta[s] * softmax(beta[s]) . (ed[ii] + ed[jj])

Sharding: S=8192 stages split across 8 cores (pure data parallel,
1024 stages/core as 128 partitions x 8 free slots). Per-core partial
losses are summed on the host.

v2 layout/engine choices:
  - alpha staged in HBM as 7 op-major bf16 planes (op 7 unused),
    node-grouped, streamed on the sync HWDGE ring (SWDGE cast-DMA from
    fp8 measured ~2x slower and its descriptor rings degrade DVE 2x).
  - max-of-7 as a 4-instruction bf16 tensor_tensor max tree (2x mode)
    instead of a 1x tensor_reduce.
  - per-node-group pipeline: tree/exp/softmax/DP for nodes [16g,16g+16)
    overlap the next group's plane DMA.
  - the softmax reciprocal is broadcast-expanded on the scalar engine so
    the normalize multiply runs dense bf16 at DVE 2x.
  - beta rides the scalar-engine HWDGE ring in bf16; its exps/matmuls
    are emitted after the group loop so they fill scalar-engine gaps.
"""

import numpy as np

SW = 16          # DP window
NN = 64          # nodes per stage
S = 8192         # stages
E = 2016         # beta edges
P = 128          # SBUF partitions
N_CORES = 8
S_CORE = S // N_CORES        # 1024
T = S_CORE // P              # 8 stage slots per partition
NG = 4                       # node groups
GN = NN // NG                # 16 nodes per group
GW = GN * SW                 # 256 edge_max floats per stage per group
GF = T * GW                  # 2048 free elems per group tile
NPL = 7                      # op planes
EDW = 67                     # ed row stride (66 node slots + 1 pad)
NCH = 16                     # beta column chunks
ECH = E // NCH               # 126 edges per chunk
NMASK = 14                   # nodes with partially-valid windows

_CACHE = {}


def _host_consts():
    import ml_dtypes

    ii, jj = [], []
    for i in range(2, NN + 1):
        for j in range(i + 1, NN + 2):
            ii.append(i)
            jj.append(j)
    ii = np.asarray(ii)
    jj = np.asarray(jj)
    # incidence matrix chunks: mt[e_local, c*67 + k] = [ii==k] + [jj==k],
    # column 66 of each chunk is all ones (softmax denominator)
    mt = np.zeros((NCH, ECH, EDW), np.float32)
    for e in range(E):
        c, el = divmod(e, ECH)
        mt[c, el, ii[e]] += 1.0
        mt[c, el, jj[e]] += 1.0
        mt[c, el, EDW - 1] = 1.0
    mt = np.ascontiguousarray(
        mt.transpose(1, 0, 2).reshape(ECH, NCH * EDW)
    ).astype(ml_dtypes.bfloat16)
    # validity mask for the first 14 nodes (node n: rows k < n+2 valid)
    mask = np.zeros((NMASK, SW), np.float32)
    for n in range(NMASK):
        mask[n, : n + 2] = 1.0
    mask = np.ascontiguousarray(
        np.broadcast_to(mask.reshape(1, NMASK * SW), (P, NMASK * SW))
    ).astype(ml_dtypes.bfloat16)
    return mt, mask


def _install_tile_patches():
    import concourse.mybir as mybir
    from concourse.tile import TileContext
    from concourse.vector_clock import ScopedClock, VectorClock

    # This walrus build rejects TPB instructions carrying more than one sem
    # wait (two for EventSemaphore, zero for Pool-engine non-ES ops), but
    # Tile's wait assignment happily packs 2-3. Split the extras onto
    # single-wait NoOps (ES chunks for Pool) on the same engine.
    if not getattr(TileContext, "_ant_wait_split", False):
        _orig_commit = TileContext._commit_instruction

        def _commit_split(self, inst, lazy_reg_writes=True):
            si = inst.sync_info
            is_es = isinstance(inst, mybir.InstEventSemaphore)
            is_pool = inst.engine == mybir.EngineType.Pool
            limit = 2 if is_es else (0 if is_pool else 1)
            if si is not None and si.on_wait and len(si.on_wait) > limit:
                waits = list(si.on_wait)
                extras = waits[: len(waits) - limit]
                if is_pool:
                    for i in range(0, len(extras), 2):
                        es = mybir.InstEventSemaphore(
                            name=f"{inst.name}-sw{i}",
                            sync_info=mybir.SyncInfo(
                                on_wait=extras[i : i + 2], on_update=[]
                            ),
                            engine=inst.engine,
                        )
                        _orig_commit(self, es, lazy_reg_writes)
                else:
                    for i, w in enumerate(extras):
                        nop = mybir.InstNoOp(
                            name=f"{inst.name}-sw{i}",
                            sync_info=mybir.SyncInfo(on_wait=[w], on_update=[]),
                            bass_nofuse=True,
                            engine=inst.engine,
                        )
                        _orig_commit(self, nop, lazy_reg_writes)
                inst.sync_info = mybir.SyncInfo(
                    on_wait=waits[len(waits) - limit :], on_update=list(si.on_update)
                )
            return _orig_commit(self, inst, lazy_reg_writes)

        TileContext._commit_instruction = _commit_split
        TileContext._ant_wait_split = True

    # The stock TileContext tail drain packs every outstanding sem wait into
    # a single InstDrain; this walrus caps non-EventSemaphore instructions at
    # one wait. Emit one drain per outstanding semaphore instead.
    def _drain_and_barrier(self, tick_clock, wait_clock):
        nc = self.nc
        gc = tick_clock.global_clock
        n = len(gc)
        for i in range(n):
            t = gc[i]
            if t <= 0:
                continue
            vc = VectorClock([0] * n)
            vc.require_at_least(i, t)
            d = nc.sync.drain()
            wait_clock.add_sem_waits(d.ins, ScopedClock({None: vc}))
        nc.all_engine_barrier()
        assert self.sems is not None
        popped = nc._tile_sem_poison_stack.pop()
        assert popped is self._sem_poison
        nc.clear_and_free_semaphores(list(self.sems.allocated().values()))
        nc.all_engine_barrier()

    TileContext._drain_and_barrier = _drain_and_barrier


def _build_nc():
    import concourse.bass as bass
    import concourse.mybir as mybir
    from concourse.tile import TileContext

    _install_tile_patches()

    f32 = mybir.dt.float32
    bf16 = mybir.dt.bfloat16
    f8 = mybir.dt.float8e4
    Alu = mybir.AluOpType
    Act = mybir.ActivationFunctionType
    X = mybir.AxisListType

    nc = bass.Bass()
    # alpha planes: row g*128+p, free [o(7), t(8), nl(16), k(16)] bf16
    alpha_d = nc.declare_dram_parameter(
        "alpha_p", [NG * P, NPL * GF], bf16, isOutput=False
    )
    # beta pre-transposed on the host into chunk layout:
    # beta_t[el, t*2048 + c*128 + p] = beta[t*128 + p, c*126 + el]
    beta_d = nc.declare_dram_parameter("beta_t", [ECH, T * NCH * P], bf16, isOutput=False)
    theta_d = nc.declare_dram_parameter("theta_t", [P, T], f32, isOutput=False)
    mt_d = nc.declare_dram_parameter("mt_c", [ECH, NCH * EDW], bf16, isOutput=False)
    out_d = nc.declare_dram_parameter("loss_part", [1, 1], f32, isOutput=True)

    with TileContext(nc) as tc:
        with (
            tc.tile_pool(name="consts", bufs=1) as cp,
            tc.tile_pool(name="planes", bufs=3) as plp,
            tc.tile_pool(name="tree1", bufs=1) as trp1,
            tc.tile_pool(name="tree2", bufs=2) as trp2,
            tc.tile_pool(name="persist", bufs=1) as pp,
            tc.tile_pool(name="smallp", bufs=2) as sp,
            tc.tile_pool(name="finp", bufs=1) as fp_,
            tc.tile_pool(name="betap", bufs=1) as bp,
            tc.tile_pool(name="ebtp", bufs=2) as ep,
            tc.tile_pool(name="psc", bufs=2, space="PSUM") as psc,
        ):
            # first plane-group DMA gates the DVE pipeline; split it so the
            # tree's first ops can start on the front half
            pl_tiles = [
                plp.tile([P, NPL * GF], bf16, tag="pl", name=f"pl{i}")
                for i in range(3)
            ]
            nc.sync.dma_start(
                pl_tiles[0][:, 0 : 2 * GF], alpha_d[0:P, 0 : 2 * GF]
            )
            nc.sync.dma_start(
                pl_tiles[0][:, 2 * GF : 4 * GF], alpha_d[0:P, 2 * GF : 4 * GF]
            )
            nc.sync.dma_start(
                pl_tiles[0][:, 4 * GF : NPL * GF], alpha_d[0:P, 4 * GF : NPL * GF]
            )

            mt_sb = cp.tile([ECH, NCH * EDW], bf16)
            nc.scalar.dma_start(mt_sb[:, :], mt_d[:, :])
            theta_sb = cp.tile([P, T], f32)
            nc.scalar.dma_start(theta_sb[:, :], theta_d[:, :])
            ones_sb = cp.tile([P, 1], f32)
            nc.vector.memset(ones_sb[:, :], 1.0)

            # prefetch groups 1-2 + the first beta tiles
            nc.sync.dma_start(pl_tiles[1][:, :], alpha_d[P : 2 * P, :])
            nc.sync.dma_start(pl_tiles[2][:, :], alpha_d[2 * P : 3 * P, :])
            b_tiles = [
                bp.tile([ECH, NCH * P], bf16, tag=f"b{t}", name=f"bt{t}")
                for t in range(T)
            ]
            for t in range(T):
                nc.sync.dma_start(
                    b_tiles[t][:, :], beta_d[:, t * NCH * P : (t + 1) * NCH * P]
                )

            w_sb = pp.tile([P, NG * GF], bf16)    # softmax weights, grouped
            ed_sb = pp.tile([P, T * EDW], f32)    # DP state, zero-init
            tmp_sb = pp.tile([P, T * SW], f32)    # DP step scratch
            nc.vector.memset(ed_sb[:, :], 0.0)

            ed3 = ed_sb.rearrange("p (t k) -> p t k", t=T)
            # softmax weights sum to 1 over zero-depth preds => ed[2] = 1
            nc.vector.memset(ed3[:, :, 2:3], 1.0)
            tmp3 = tmp_sb.rearrange("p (t k) -> p t k", k=SW)

            c_ps = psc.tile([P, T * EDW], f32, tag="c", bufs=1)

            for g in range(NG):
                pl = pl_tiles[g % 3]
                pv = pl.rearrange("p (o f) -> p o f", o=NPL)

                # max tree over 7 planes, all operands dense bf16 (2x mode)
                mxg = trp2.tile([P, GF], bf16, tag="mx")
                if g == 0:
                    # chase the three g0 DMA pieces
                    lb = trp1.tile([P, GF], bf16, tag="lb")
                    nc.vector.tensor_tensor(lb[:, :], pv[:, 0, :], pv[:, 1, :], Alu.max)
                    la = trp1.tile([P, GF], bf16, tag="la")
                    nc.vector.tensor_tensor(la[:, :], pv[:, 2, :], pv[:, 3, :], Alu.max)
                    nc.vector.tensor_tensor(lb[:, :], lb[:, :], la[:, :], Alu.max)
                    lc = trp1.tile([P, GF], bf16, tag="lc")
                    nc.vector.tensor_tensor(lc[:, :], pv[:, 4, :], pv[:, 5, :], Alu.max)
                    nc.vector.tensor_tensor(lc[:, :], lc[:, :], pv[:, 6, :], Alu.max)
                    nc.vector.tensor_tensor(mxg[:, :], lb[:, :], lc[:, :], Alu.max)
                else:
                    # L1: max(planes 0-2, planes 3-5) in one dense op
                    la = trp1.tile([P, 3 * GF], bf16, tag="la")
                    l3 = la.rearrange("p (i f) -> p i f", i=3)
                    nc.vector.tensor_tensor(
                        l3[:, :, :], pv[:, 0:3, :], pv[:, 3:6, :], Alu.max
                    )
                    lb = trp1.tile([P, GF], bf16, tag="lb")
                    nc.vector.tensor_tensor(
                        lb[:, :], l3[:, 0, :], l3[:, 1, :], Alu.max
                    )
                    lc = trp1.tile([P, GF], bf16, tag="lc")
                    nc.vector.tensor_tensor(lc[:, :], l3[:, 2, :], pv[:, 6, :], Alu.max)
                    nc.vector.tensor_tensor(mxg[:, :], lb[:, :], lc[:, :], Alu.max)

                # reuse buffer 0 for group 3 once group 0's tree is done
                if g == 0:
                    nc.sync.dma_start(
                        pl_tiles[0][:, :], alpha_d[3 * P : 4 * P, :]
                    )

                # softmax numerator without max-subtraction (|logits| <~ 6)
                e_sl = w_sb[:, g * GF : (g + 1) * GF]
                nc.scalar.activation(
                    e_sl[:, 0 : GF // 2], mxg[:, 0 : GF // 2], Act.Exp
                )
                nc.scalar.activation(
                    e_sl[:, GF // 2 : GF], mxg[:, GF // 2 : GF], Act.Exp
                )

                # two bf16 pair-add levels at 2x, then a 4-wide 1x reduce
                ph = sp.tile([P, T * GN * 8], bf16, tag="ph")
                e4 = e_sl.rearrange("p (n k) -> p n k", k=SW)
                nc.vector.tensor_add(
                    ph.rearrange("p (n k) -> p n k", k=8),
                    e4[:, :, 0:8],
                    e4[:, :, 8:16],
                )
                pq = sp.tile([P, T * GN * 4], bf16, tag="pq")
                p8 = ph.rearrange("p (n k) -> p n k", k=8)
                nc.vector.tensor_add(
                    pq.rearrange("p (n k) -> p n k", k=4),
                    p8[:, :, 0:4],
                    p8[:, :, 4:8],
                )
                s_g = sp.tile([P, T * GN], f32, tag="s")
                nc.vector.reduce_sum(
                    s_g[:, :], pq.rearrange("p (n k) -> p n k", k=4), axis=X.X
                )
                lns = sp.tile([P, T * GN], f32, tag="lns")
                nc.scalar.activation(lns[:, :], s_g[:, :], Act.Ln)
                # fused exp(-ln s) + broadcast-expand to [., n, 16] on ACT so
                # the normalize multiply below runs dense bf16 at 2x
                rse = sp.tile([P, GF], bf16, tag="rse")
                nc.scalar.activation(
                    rse.rearrange("p (n k) -> p n k", k=SW),
                    lns.rearrange("p (n o) -> p n o", o=1).broadcast_to(
                        (P, T * GN, SW)
                    ),
                    Act.Exp,
                    scale=-1.0,
                )
                nc.vector.tensor_mul(e_sl, e_sl, rse[:, :])

                # DP steps for this group's nodes (all 8 stage slots at once)
                wg = w_sb[:, g * GF : (g + 1) * GF].rearrange(
                    "p (t n k) -> p t n k", t=T, k=SW
                )
                if g == 0:
                    # ed[3] = 1 + w[3,2]*ed[2] = 1 + w[3,2] (scalar engine)
                    nc.scalar.add(ed3[:, :, 3:4], wg[:, :, 1, 2:3], 1.0)
                for nl in range(2 if g == 0 else 0, GN):
                    j = g * GN + nl + 2
                    wid = min(j, SW)
                    base = j - wid
                    nc.vector.scalar_tensor_tensor(
                        tmp3[:, :, 0:wid],
                        ed3[:, :, base : base + wid],
                        1.0,
                        wg[:, :, nl, 0:wid],
                        Alu.add,
                        Alu.mult,
                    )
                    nc.vector.reduce_sum(
                        ed3[:, :, j : j + 1], tmp3[:, :, 0:wid], axis=X.X
                    )

            # ---- beta phase (low priority: fills scalar/PE gaps) ----
            for t in range(T):
                eb_t = ep.tile([ECH, NCH * P], bf16, tag="eb")
                half = NCH * P // 2
                nc.scalar.activation(
                    eb_t[:, 0:half], b_tiles[t][:, 0:half], Act.Exp
                )
                nc.scalar.activation(
                    eb_t[:, half:], b_tiles[t][:, half:], Act.Exp
                )
                for c in range(NCH):
                    nc.tensor.matmul(
                        c_ps[:, t * EDW : (t + 1) * EDW],
                        eb_t[:, c * P : (c + 1) * P],
                        mt_sb[:, c * EDW : (c + 1) * EDW],
                        start=(c == 0),
                        stop=(c == NCH - 1),
                    )

            # ---- final dots: batched over all 8 stage slots ----
            prod = fp_.tile([P, T * (EDW - 1)], f32, tag="prod")
            q = fp_.tile([P, T], f32, tag="q")
            cv = c_ps.rearrange("p (t k) -> p t k", k=EDW)
            nc.vector.scalar_tensor_tensor(
                prod.rearrange("p (t k) -> p t k", k=EDW - 1),
                ed3[:, :, 0 : EDW - 1],
                0.0,
                cv[:, :, 0 : EDW - 1],
                Alu.add,
                Alu.mult,
            )
            nc.vector.reduce_sum(
                q.rearrange("p (t k) -> p t k", k=1),
                prod.rearrange("p (t k) -> p t k", k=EDW - 1),
                axis=X.X,
            )
            rsb = fp_.tile([P, T], f32, tag="rsb")
            nc.vector.reciprocal(rsb[:, :], cv[:, :, EDW - 1])
            rst = fp_.tile([P, T], f32, tag="rst")
            nc.vector.tensor_mul(rst[:, :], rsb[:, :], theta_sb[:, :])
            acc = fp_.tile([P, T], f32, tag="acc")
            nc.vector.tensor_mul(acc[:, :], q[:, :], rst[:, :])

            # ---- final reduction: 8 cols then 128 partitions ----
            accsum = fp_.tile([P, 1], f32, tag="accsum")
            nc.vector.reduce_sum(accsum[:, :], acc[:, :], axis=X.X)
            out_ps = psc.tile([1, 1], f32, tag="outp", bufs=1)
            nc.tensor.matmul(
                out_ps[:, :], accsum[:, :], ones_sb[:, :], start=True, stop=True
            )
            out_sb = fp_.tile([1, 1], f32, tag="outs")
            nc.scalar.copy(out_sb[:, :], out_ps[:, :])
            nc.sync.dma_start(out_d[:, :], out_sb[:, :])

    return nc


def _get_compiled():
    if "nc" not in _CACHE:
        _CACHE["nc"] = _build_nc()
        _CACHE["consts"] = _host_consts()
    return _CACHE["nc"], _CACHE["consts"]


def _in_maps(alpha, beta, theta):
    import ml_dtypes

    mt, mask = _get_compiled()[1]
    alpha = np.ascontiguousarray(alpha, dtype=np.float32)
    beta = np.ascontiguousarray(beta, dtype=np.float32)
    theta = np.ascontiguousarray(theta, dtype=np.float32)
    alpha_bf = alpha.astype(ml_dtypes.bfloat16)
    beta_bf = beta.astype(ml_dtypes.bfloat16)
    maps = []
    for c in range(N_CORES):
        sl = slice(c * S_CORE, (c + 1) * S_CORE)
        # [t, p, g, nl, k, o] -> [g, p, o, t, nl, k], drop op 7
        A = alpha_bf[sl].reshape(T, P, NG, GN, SW, 8)
        planes = np.ascontiguousarray(A.transpose(2, 1, 5, 0, 3, 4)[:, :, :NPL])
        # bake the window-validity mask into group 0's padding slots:
        # node n has valid rows k < n+2; exp(-300) underflows to 0
        inv = np.zeros((GN, SW), bool)
        for n in range(NMASK):
            inv[n, n + 2 :] = True
        planes[0][:, :, :, inv] = np.float32(-300.0).astype(planes.dtype)
        # [el, t*2048 + ch*128 + p] = beta[t*128 + p, ch*126 + el]
        beta_t = np.ascontiguousarray(
            beta_bf[sl].reshape(T, P, NCH, ECH).transpose(3, 0, 2, 1).reshape(ECH, -1)
        )
        maps.append(
            {
                "alpha_p": planes.reshape(NG * P, NPL * GF),
                "beta_t": beta_t,
                "theta_t": np.ascontiguousarray(theta[sl].reshape(T, P).T),
                "mt_c": mt,
            }
        )
    return maps


def _run(alpha, beta, theta, **spmd_kwargs):
    from concourse.bass_utils import run_bass_kernel_spmd

    nc, _ = _get_compiled()
    res = run_bass_kernel_spmd(
        nc, _in_maps(alpha, beta, theta), core_ids=list(range(N_CORES)), **spmd_kwargs
    )
    total = np.float32(0.0)
    for r in res.results:
        total += np.float32(r["loss_part"][0, 0])
    return np.float32(total), res


def kernel(alpha, beta, theta):
    out, _ = _run(alpha, beta, theta)
    return out



# revision 2
# speedup vs baseline: 1.0567x; 1.0567x over previous
"""Trainium2 Bass kernel v3 for the Expected-Depth DP loss.

Structure (per core, 1024 stages as 128 partitions x 8 slots):
  - alpha planes streamed per node-group of 8 nodes (8 groups), op-major
    bf16, HWDGE sync ring, 2 pieces per group (planes 0-5, plane 6).
  - max-of-7 tree on DVE per group (4 ops).
  - softmax: exp (ACT) -> 2 pair-adds + reduce (DVE) -> ln (ACT) ->
    exp(-ln s) broadcast-expand (ACT) -> normalize mul (DVE).
  - DP scan: uniform full-16 windows against a zero-padded ed row
    (invalid taps have w=0 from the -300 logit bake).  DP_MODE selects
    the engine: "pool" (STT + 4 pair-add halvings on GpSimd),
    "dve_scan" (custom fused scan op + boundary-diff), or
    "dve_classic" (STT + reduce, as v2).
  - beta: exp per t-tile (ACT), 16 matmuls per t with mt as the
    stationary operand, PSUM -> bf16 copy (ACT), PE transpose back to
    stage-major, final dots on DVE as v2.
"""

import numpy as np

SW = 16
NN = 64
S = 8192
E = 2016
P = 128
N_CORES = 8
S_CORE = S // N_CORES          # 1024
T = S_CORE // P                # 8
NGRP = 4
GN = NN // NGRP                # 16 nodes per group
GF = T * GN * SW               # 2048 free elems per plane per group
# (node_start, width) processing sections; first two are halves of the
# node range so the pipeline starts on a 1.8 MB DMA instead of 3.7 MB
SECTIONS = [(0, 8), (8, 8), (16, 16), (32, 16), (48, 16)]
NPL = 7
EDP = 16                       # ed row left zero-padding
EDW = EDP + NN + 3             # 83: padded ed row width
NCH = 16
ECH = E // NCH                 # 126
NMASK = 14

DP_MODE = "dve_scan"           # "pool" | "dve_scan" | "dve_classic"
TREE_L1_POOL = False           # run the wide first max level on GpSimd
SCAN_SHA = "299927bc89edaa07"

_CACHE = {}


def _host_consts():
    import ml_dtypes

    ii, jj = [], []
    for i in range(2, NN + 1):
        for j in range(i + 1, NN + 2):
            ii.append(i)
            jj.append(j)
    ii = np.asarray(ii)
    jj = np.asarray(jj)
    mt = np.zeros((NCH, ECH, 67), np.float32)
    for e in range(E):
        c, el = divmod(e, ECH)
        mt[c, el, ii[e]] += 1.0
        mt[c, el, jj[e]] += 1.0
        mt[c, el, 66] = 1.0
    mt = np.ascontiguousarray(
        mt.transpose(1, 0, 2).reshape(ECH, NCH * 67)
    ).astype(ml_dtypes.bfloat16)
    ident = np.eye(P, dtype=np.float32).astype(ml_dtypes.bfloat16)
    return mt, ident


def _install_tile_patches():
    import concourse.mybir as mybir
    from concourse.tile import TileContext
    from concourse.vector_clock import ScopedClock, VectorClock

    # This walrus build rejects TPB instructions carrying more than one sem
    # wait (two for EventSemaphore, zero for Pool-engine non-ES ops), but
    # Tile's wait assignment happily packs 2-3. Split the extras onto
    # single-wait NoOps (ES chunks for Pool) on the same engine.
    if not getattr(TileContext, "_ant_wait_split", False):
        _orig_commit = TileContext._commit_instruction

        def _commit_split(self, inst, lazy_reg_writes=True):
            si = inst.sync_info
            is_es = isinstance(inst, mybir.InstEventSemaphore)
            is_pool = inst.engine == mybir.EngineType.Pool
            limit = 2 if is_es else (0 if is_pool else 1)
            if si is not None and si.on_wait and len(si.on_wait) > limit:
                waits = list(si.on_wait)
                extras = waits[: len(waits) - limit]
                if is_pool:
                    for i in range(0, len(extras), 2):
                        es = mybir.InstEventSemaphore(
                            name=f"{inst.name}-sw{i}",
                            sync_info=mybir.SyncInfo(
                                on_wait=extras[i : i + 2], on_update=[]
                            ),
                            engine=inst.engine,
                        )
                        _orig_commit(self, es, lazy_reg_writes)
                else:
                    for i, w in enumerate(extras):
                        nop = mybir.InstNoOp(
                            name=f"{inst.name}-sw{i}",
                            sync_info=mybir.SyncInfo(on_wait=[w], on_update=[]),
                            bass_nofuse=True,
                            engine=inst.engine,
                        )
                        _orig_commit(self, nop, lazy_reg_writes)
                inst.sync_info = mybir.SyncInfo(
                    on_wait=waits[len(waits) - limit :], on_update=list(si.on_update)
                )
            return _orig_commit(self, inst, lazy_reg_writes)

        TileContext._commit_instruction = _commit_split
        TileContext._ant_wait_split = True

    # The stock TileContext tail drain packs every outstanding sem wait into
    # a single InstDrain; this walrus caps non-EventSemaphore instructions at
    # one wait. Emit one drain per outstanding semaphore instead.
    def _drain_and_barrier(self, tick_clock, wait_clock):
        nc = self.nc
        gc = tick_clock.global_clock
        n = len(gc)
        for i in range(n):
            t = gc[i]
            if t <= 0:
                continue
            vc = VectorClock([0] * n)
            vc.require_at_least(i, t)
            d = nc.sync.drain()
            wait_clock.add_sem_waits(d.ins, ScopedClock({None: vc}))
        nc.all_engine_barrier()
        assert self.sems is not None
        popped = nc._tile_sem_poison_stack.pop()
        assert popped is self._sem_poison
        nc.clear_and_free_semaphores(list(self.sems.allocated().values()))
        nc.all_engine_barrier()

    TileContext._drain_and_barrier = _drain_and_barrier


def _register_scan_op():
    import concourse.dve_ops as do
    import concourse.dve_spec as ds
    from concourse.dve_spec import Spec, Src0, Src1, AluOp, Scan, One

    for o in do.OPS:
        if o.name == "DP_SEG_SCAN_ANT":
            return o

    # segmented-scan support: step override re-seeds from init at each
    # SUB_DIM_DONE (inner-dim wrap of the [P,S,N] pattern)
    if not getattr(ds, "_ant_seg_patch", False):
        _orig = ds._scan_overrides

        def _scan_ov_seg(scans, node_stage):
            seed, step = _orig(scans, node_stage)
            for sc_ in scans:
                if getattr(sc_, "_ant_seg", False):
                    d = node_stage[sc_]
                    step[d] = ds._Stage(sc_.op, ds._scan_init(sc_), sc_.expr)
            return seed, step

        ds._scan_overrides = _scan_ov_seg
        ds._ant_seg_patch = True

    def ref(in0, in1, s0, s1, imm2):
        b = ((in0.astype(np.float32) + 1.0) * in1).astype(np.float32)
        return np.cumsum(b, axis=-1)

    sc = Scan(AluOp.ADD, (Src0 + One) * Src1)
    object.__setattr__(sc, "_ant_seg", True)
    op = do.DveOp(
        "DP_SEG_SCAN_ANT",
        Spec(body=sc, reference=ref),
        subdim=True,
        uops_sha={"v3": SCAN_SHA},
    )
    do.OPS.append(op)
    do.CUSTOM_DVE_SPECS[op.name] = op.spec
    do._SUB_OPCODE_FOR_NAME[op.name] = do._CUSTOM_DVE_ROW_BASE + len(do.OPS) - 1
    return op


def _build_nc():
    import concourse.bass as bass
    import concourse.mybir as mybir
    from concourse.tile import TileContext

    _install_tile_patches()
    scan_op = _register_scan_op() if DP_MODE == "dve_scan" else None

    f32 = mybir.dt.float32
    bf16 = mybir.dt.bfloat16
    Alu = mybir.AluOpType
    Act = mybir.ActivationFunctionType
    X = mybir.AxisListType

    nc = bass.Bass()
    # alpha planes: [P, g-concat of (o(7), t(8), nl(8), k(16))] bf16
    alpha_d = nc.declare_dram_parameter(
        "alpha_p", [P, NGRP * NPL * GF], bf16, isOutput=False
    )
    # beta pre-transposed: beta_t[el, t*2048 + c*128 + p] = beta[t*128+p, c*126+el]
    beta_d = nc.declare_dram_parameter(
        "beta_t", [ECH, T * NCH * P], bf16, isOutput=False
    )
    theta_d = nc.declare_dram_parameter("theta_t", [P, T], f32, isOutput=False)
    mt_d = nc.declare_dram_parameter("mt_c", [ECH, NCH * 67], bf16, isOutput=False)
    id_d = nc.declare_dram_parameter("ident", [P, P], bf16, isOutput=False)
    out_d = nc.declare_dram_parameter("loss_part", [1, 1], f32, isOutput=True)

    with TileContext(nc) as tc:
        with (
            tc.tile_pool(name="consts", bufs=1) as cp,
            tc.tile_pool(name="planes", bufs=2) as plp,
            tc.tile_pool(name="planes2", bufs=2) as plp2,
            tc.tile_pool(name="tree1", bufs=1) as trp1,
            tc.tile_pool(name="tree2", bufs=2) as trp2,
            tc.tile_pool(name="persist", bufs=1) as pp,
            tc.tile_pool(name="smallp", bufs=2) as sp,
            tc.tile_pool(name="smallp1", bufs=1) as sp1,
            tc.tile_pool(name="finp", bufs=1) as fp_,
            tc.tile_pool(name="betap", bufs=6) as bp,
            tc.tile_pool(name="ebtp", bufs=3) as ep,
            tc.tile_pool(name="psc", bufs=1, space="PSUM") as psc,
            tc.tile_pool(name="psc2", bufs=1, space="PSUM") as psc2,
        ):
            mt_sb = cp.tile([ECH, NCH * 67], bf16)
            theta_sb = cp.tile([P, T], f32)
            id_sb = cp.tile([P, P], bf16)
            ones_sb = cp.tile([P, 1], f32)
            nc.vector.memset(ones_sb[:, :], 1.0)

            b_tiles = {}
            eb_tiles = {}

            def emit_beta_dma(t):
                bt = bp.tile([ECH, NCH * P], bf16, tag="b", name=f"bt{t}")
                b_tiles[t] = bt
                nc.sync.dma_start(
                    bt[:, :], beta_d[:, t * NCH * P : (t + 1) * NCH * P]
                )

            # plane DMAs: one contiguous DMA per section
            pl_tiles = []
            off = 0
            for si, (n0, wdt) in enumerate(SECTIONS):
                fs = NPL * T * wdt * SW
                pool = plp if wdt == 16 else plp2
                t_ = pool.tile(
                    [P, fs], bf16, tag=f"pl{wdt}", name=f"pl{si}"
                )
                pl_tiles.append(t_)
                nc.sync.dma_start(t_[:, :], alpha_d[:, off : off + fs])
                off += fs
                if si == 2:
                    for bt_ in (0, 1, 2, 3):
                        emit_beta_dma(bt_)
                if si == 3:
                    for bt_ in (4, 5):
                        emit_beta_dma(bt_)
                if si == 4:
                    for bt_ in (6, 7):
                        emit_beta_dma(bt_)
                if si == 1:
                    # small consts ride between the alpha sections
                    nc.sync.dma_start(mt_sb[:, :], mt_d[:, :])
                    nc.sync.dma_start(theta_sb[:, :], theta_d[:, :])
                    nc.sync.dma_start(id_sb[:, :], id_d[:, :])

            w_sb = pp.tile([P, NGRP * GF], bf16)   # softmax weights
            ed_sb = pp.tile([P, T * EDW], f32)     # padded DP state
            nc.vector.memset(ed_sb[:, :], 0.0)
            ed3 = ed_sb.rearrange("p (t k) -> p t k", t=T)

            if DP_MODE in ("pool", "dve_classic"):
                tmp_sb = pp.tile([P, T * SW], f32)
                tmp3 = tmp_sb.rearrange("p (t k) -> p t k", k=SW)
            if DP_MODE == "pool":
                onep = pp.tile([P, T], f32)
                nc.gpsimd.memset(onep[:, :], 1.0)
                o3 = onep.rearrange("p (t k) -> p t k", k=1)

            def dp_step(j, wv):
                """DP step for absolute node index j (2..65); wv is the
                node's weight row view [P, T, SW]."""
                win = ed3[:, :, j - 16 + EDP : j + EDP]
                dst = ed3[:, :, j + EDP : j + EDP + 1]
                if DP_MODE == "pool":
                    # w rows sum to 1, so ed[j] = sum(w*ed) + 1
                    nc.gpsimd.tensor_tensor(tmp3[:, :, :], win, wv, Alu.mult)
                    for h in (8, 4, 2):
                        nc.gpsimd.tensor_tensor(
                            tmp3[:, :, 0:h], tmp3[:, :, 0:h],
                            tmp3[:, :, h : 2 * h], Alu.add,
                        )
                    nc.gpsimd.tensor_tensor(
                        tmp3[:, :, 0:1], tmp3[:, :, 0:1], tmp3[:, :, 1:2],
                        Alu.add,
                    )
                    nc.gpsimd.tensor_tensor(
                        dst, tmp3[:, :, 0:1], o3[:, :, :], Alu.add
                    )
                elif DP_MODE == "dve_classic":
                    nc.vector.scalar_tensor_tensor(
                        tmp3[:, :, :], win, 1.0, wv, Alu.add, Alu.mult
                    )
                    nc.vector.reduce_sum(dst, tmp3[:, :, :], axis=X.X)
                else:
                    nc.vector._custom_dve(
                        scan_op,
                        out=dst.broadcast_to((P, T, SW)),
                        in0=win,
                        in1=wv,
                    )


            def emit_beta_exp(t):
                eb = ep.tile([ECH, NCH * P], bf16, tag="e", name=f"ebt{t}")
                eb_tiles[t] = eb
                nc.scalar.activation(eb[:, :], b_tiles[t][:, :], Act.Exp)

            c_ps = psc.tile([67, T * P], f32, tag="c", bufs=1)
            c_sb = fp_.tile([67, T * P], bf16, tag="csb")

            # ---- main section loop ----
            beta_exp_plan = {2: [0, 1], 3: [2, 3]}
            CARRY = 6
            pending = []  # carried DP steps from the previous section

            for si, (n0, wdt) in enumerate(SECTIONS):
                pl = pl_tiles[si]
                fs = T * wdt * SW
                pv = pl.rearrange("p (o f) -> p o f", o=NPL)

                if si == len(SECTIONS) - 1:
                    # flush the beta tail (ACT exps, PE matmuls, PSUM
                    # copies) before the last ladder so the chain
                    # overlaps the remaining DP work
                    for t in (4, 5, 6, 7):
                        emit_beta_exp(t)
                    for t in range(T):
                        for c in range(NCH):
                            nc.tensor.matmul(
                                c_ps[:, t * P : (t + 1) * P],
                                mt_sb[:, c * 67 : (c + 1) * 67],
                                eb_tiles[t][:, c * P : (c + 1) * P],
                                start=(c == 0),
                                stop=(c == NCH - 1),
                            )
                    for t in range(T):
                        nc.scalar.copy(
                            c_sb[:, t * P : (t + 1) * P],
                            c_ps[:, t * P : (t + 1) * P],
                        )

                # max tree
                l3 = trp1.tile([P, 3 * GF], bf16, tag="l3")
                l33 = l3[:, 0 : 3 * fs].rearrange("p (i f) -> p i f", i=3)
                nc.vector.tensor_tensor(
                    l33[:, :, :], pv[:, 0:3, :], pv[:, 3:6, :], Alu.max
                )
                lb = trp1.tile([P, GF], bf16, tag="lb")
                nc.vector.tensor_tensor(
                    lb[:, 0:fs], l33[:, 0, :], l33[:, 1, :], Alu.max
                )
                lc = trp1.tile([P, GF], bf16, tag="lc")
                nc.vector.tensor_tensor(
                    lc[:, 0:fs], l33[:, 2, :], pv[:, 6, :], Alu.max
                )
                mxg = trp2.tile([P, GF], bf16, tag="mx")
                nc.vector.tensor_tensor(
                    mxg[:, 0:fs], lb[:, 0:fs], lc[:, 0:fs], Alu.max
                )

                # carried DP steps of the previous section fill the
                # exp round-trip bubble on the DVE queue
                for j_, wv_ in pending:
                    dp_step(j_, wv_)
                pending = []

                # softmax numerator (no max-subtraction; |logits| small)
                e_sl = w_sb[:, n0 * T * SW : n0 * T * SW + fs]
                nc.scalar.activation(e_sl, mxg[:, 0:fs], Act.Exp)

                # window sums: 2 bf16 pair-add levels + 4-wide reduce
                nn_ = T * wdt
                e4 = e_sl.rearrange("p (n k) -> p n k", k=SW)
                ph = sp1.tile([P, T * GN * 8], bf16, tag="ph")
                nc.vector.tensor_add(
                    ph[:, 0 : nn_ * 8].rearrange("p (n k) -> p n k", k=8),
                    e4[:, :, 0:8],
                    e4[:, :, 8:16],
                )
                pq = sp1.tile([P, T * GN * 4], bf16, tag="pq")
                p8 = ph[:, 0 : nn_ * 8].rearrange("p (n k) -> p n k", k=8)
                nc.vector.tensor_add(
                    pq[:, 0 : nn_ * 4].rearrange("p (n k) -> p n k", k=4),
                    p8[:, :, 0:4],
                    p8[:, :, 4:8],
                )
                s_g = sp.tile([P, T * GN], f32, tag="s")
                nc.vector.reduce_sum(
                    s_g[:, 0:nn_],
                    pq[:, 0 : nn_ * 4].rearrange("p (n k) -> p n k", k=4),
                    axis=X.X,
                )
                lns = sp.tile([P, T * GN], f32, tag="lns")
                nc.scalar.activation(lns[:, 0:nn_], s_g[:, 0:nn_], Act.Ln)
                rse = sp.tile([P, GF], bf16, tag="rse")
                nc.scalar.activation(
                    rse[:, 0:fs].rearrange("p (n k) -> p n k", k=SW),
                    lns[:, 0:nn_]
                    .rearrange("p (n o) -> p n o", o=1)
                    .broadcast_to((P, nn_, SW)),
                    Act.Exp,
                    scale=-1.0,
                )
                nc.vector.tensor_mul(e_sl, e_sl, rse[:, 0:fs])

                # beta exps riding the ACT queue
                for t in beta_exp_plan.get(si, []):
                    emit_beta_exp(t)

                # DP steps; carry the last few into the next section
                wg3 = e_sl.rearrange("p (t n k) -> p t n k", t=T, k=SW)
                steps = [
                    (n0 + nl + 2, wg3[:, :, nl, :]) for nl in range(wdt)
                ]
                if si < len(SECTIONS) - 1:
                    pending = steps[-CARRY:]
                    steps = steps[:-CARRY]
                for j_, wv_ in steps:
                    dp_step(j_, wv_)
            for j_, wv_ in pending:
                dp_step(j_, wv_)

            # transpose back to stage-major [P, t, 67]
            c2_ps = psc2.tile([P, T * 68], bf16, tag="c2", bufs=1)
            for t in range(T):
                nc.tensor.transpose(
                    c2_ps[:, t * 68 : t * 68 + 67],
                    c_sb[:, t * P : (t + 1) * P],
                    id_sb[0:67, 0:67],
                )

            # ---- final dots over all 8 stage slots ----
            cv = c2_ps.rearrange("p (t k) -> p t k", k=68)
            prod = fp_.tile([P, T * 66], f32, tag="prod")
            nc.vector.scalar_tensor_tensor(
                prod.rearrange("p (t k) -> p t k", k=66),
                ed3[:, :, EDP : EDP + 66],
                0.0,
                cv[:, :, 0:66],
                Alu.add,
                Alu.mult,
            )
            q = fp_.tile([P, T], f32, tag="q")
            nc.vector.reduce_sum(
                q.rearrange("p (t k) -> p t k", k=1),
                prod.rearrange("p (t k) -> p t k", k=66),
                axis=X.X,
            )
            rsb = fp_.tile([P, T], f32, tag="rsb")
            nc.vector.reciprocal(rsb[:, :], cv[:, :, 66])
            rst = fp_.tile([P, T], f32, tag="rst")
            nc.vector.tensor_mul(rst[:, :], rsb[:, :], theta_sb[:, :])
            acc = fp_.tile([P, T], f32, tag="acc")
            nc.vector.tensor_mul(acc[:, :], q[:, :], rst[:, :])

            accsum = fp_.tile([P, 1], f32, tag="accsum")
            nc.vector.reduce_sum(accsum[:, :], acc[:, :], axis=X.X)
            out_ps = psc2.tile([1, 1], f32, tag="outp", bufs=1)
            nc.tensor.matmul(
                out_ps[:, :], accsum[:, :], ones_sb[:, :], start=True, stop=True
            )
            out_sb = fp_.tile([1, 1], f32, tag="outs")
            nc.scalar.copy(out_sb[:, :], out_ps[:, :])
            nc.sync.dma_start(out_d[:, :], out_sb[:, :])

    # populate .instr bytes for InstISA subclasses (custom DVE op);
    # raw Bass doesn't run this pass and walrus rejects empty instr.
    import concourse.mybir as _mybir

    _mybir.codegen_inst_isa_subclasses(nc)
    return nc


def _get_compiled():
    if "nc" not in _CACHE:
        _CACHE["nc"] = _build_nc()
        _CACHE["consts"] = _host_consts()
    return _CACHE["nc"], _CACHE["consts"]


def _in_maps(alpha, beta, theta):
    import ml_dtypes

    mt, ident = _get_compiled()[1]
    alpha = np.ascontiguousarray(alpha, dtype=np.float32)
    beta = np.ascontiguousarray(beta, dtype=np.float32)
    theta = np.ascontiguousarray(theta, dtype=np.float32)
    alpha_bf = alpha.astype(ml_dtypes.bfloat16)
    beta_bf = beta.astype(ml_dtypes.bfloat16)
    maps = []
    for core in range(N_CORES):
        sl = slice(core * S_CORE, (core + 1) * S_CORE)
        # [t, p, n, k, o] -> per group [p, o, t, nl, k], drop op 7
        A = alpha_bf[sl].reshape(T, P, NN, SW, 8)
        planes = np.ascontiguousarray(
            A[..., :NPL].transpose(1, 4, 0, 2, 3)
        )  # [p, o, t, n, k]
        # bake window-validity, right-aligned for uniform full-16 windows:
        # node n (j=n+2) has j valid rows; tap k of a full window maps to
        # pred j-16+k, so valid rows s=0..j-1 sit at k=16-j+s.
        neg = np.float32(-300.0).astype(planes.dtype)
        for n in range(NMASK):
            j = n + 2
            rows = planes[:, :, :, n, 0:j].copy()
            planes[:, :, :, n, :] = neg
            planes[:, :, :, n, SW - j :] = rows
        blocks = []
        for n0, wdt in SECTIONS:
            blk = planes[:, :, :, n0 : n0 + wdt, :]  # [p,o,t,nl,k]
            blocks.append(blk.reshape(P, NPL * T * wdt * SW))
        alpha_p = np.ascontiguousarray(np.concatenate(blocks, axis=1))
        beta_t = np.ascontiguousarray(
            beta_bf[sl].reshape(T, P, NCH, ECH).transpose(3, 0, 2, 1).reshape(ECH, -1)
        )
        maps.append(
            {
                "alpha_p": alpha_p,
                "beta_t": beta_t,
                "theta_t": np.ascontiguousarray(theta[sl].reshape(T, P).T),
                "mt_c": mt,
                "ident": ident,
            }
        )
    return maps


def _run(alpha, beta, theta, **spmd_kwargs):
    from concourse.bass_utils import run_bass_kernel_spmd

    nc, _ = _get_compiled()
    res = run_bass_kernel_spmd(
        nc, _in_maps(alpha, beta, theta), core_ids=list(range(N_CORES)), **spmd_kwargs
    )
    total = np.float32(0.0)
    for r in res.results:
        total += np.float32(r["loss_part"][0, 0])
    return np.float32(total), res


def kernel(alpha, beta, theta):
    out, _ = _run(alpha, beta, theta)
    return out
